# revision 14
# baseline (speedup 1.0000x reference)
"""Trainium2 Bass kernel for nn_AntisymmetricLayer — v7.

Math: out[n,k] = z@W^T + sum_r (z@P[k,:,r])*(s@Q[k,:,r]),  z=x1-x2, s=x1+x2.

Layout/pipeline (per core; tokens data-parallel over 8 cores):
  host   : uploads x1^T/x2^T [128 d, n] as bf16 (layout + dtype prep)
  DMA    : x^T tiles [128, 1024] bf16 plain HWDGE loads
  DVE    : z^T = x1^T - x2^T, s^T = x1^T + x2^T  (bf16 SBUF 2x mode)
  PE     : per 512-token block, 8 kr-chunks (kr = 64k x 16r = 1024):
           A pair-bank tiles [128, 2, 512] f32; B per-chunk [128, 512]
           outT = W^T matmul (lin, opens group) + sel_c^T @ prod_c
  ACT    : stage B PSUM->SBUF bf16 per chunk into pair tiles
  DVE    : prod pair = A_pair(PSUM, FD1024) * bs_pair(SBUF) -> bf16 SBUF
           osb: outT PSUM->SBUF
  skew   : sel matmuls trail their pair by 2 so PE never waits on DVE.

PSUM budget (8 banks): pa pairs 2x2 + pb 3x1 + po 1 (shared; blocks
alternate partition halves 0-63 / 64-127 — per-partition PSUM state makes
the halves independent).

out in DRAM is [K, n]; host transposes + un-permutes rows.
sel_c[p, k] = 1 iff k maps to strip row (sums groups of 16 kr-partitions);
adjacent chunks land on different 32-row col-groups (concurrent in PE).
"""

import numpy as np
import ml_dtypes

import concourse.bass as bass
import concourse.mybir as mybir
import concourse.tile as tile
from concourse import bacc
from concourse.bass import ts
from concourse.bass_utils import run_bass_kernel_spmd

F32 = mybir.dt.float32
BF16 = mybir.dt.bfloat16

D = 128
K = 64
R = 16
KR = K * R  # 1024
NCHUNK = KR // 128  # 8 kr-chunks of 128
NPAIR = NCHUNK // 2
SELW = NCHUNK * 32  # 256 (32-wide strips)
CONST_W = 2 * KR + K + SELW  # p2|q2|wt|sel
N_CORES = 8
OUT_T = True  # DRAM output is [K, n]; host transposes
TILE = 128
CHUNK_TILES = 4     # tokens per block = 512
BLK = TILE * CHUNK_TILES
XBLK = 2 * BLK      # tokens per input DMA / z,s compute = 1024
SEL_SKEW = 2        # sel matmuls trail their pair by this many pairs


def build_bass(n_tokens: int = 16384):
    xblk = min(XBLK, n_tokens)
    assert n_tokens % xblk == 0 and xblk % BLK == 0
    n_blocks = n_tokens // BLK

    nc = bacc.Bacc(None, target_bir_lowering=False)

    # host uploads transposed bf16 shards [D, n]
    x1t = nc.declare_dram_parameter("x1t", [D, n_tokens], BF16, isOutput=False)
    x2t = nc.declare_dram_parameter("x2t", [D, n_tokens], BF16, isOutput=False)
    cw = nc.declare_dram_parameter("cw", [D, CONST_W], BF16, isOutput=False)
    # output stored transposed [K, n]; host transposes after gather
    out = nc.declare_dram_parameter("out", [K, n_tokens], F32, isOutput=True)

    with tile.TileContext(nc) as tc:
        with (
            tc.tile_pool(name="const", bufs=1) as cpool,
            tc.tile_pool(name="xin", bufs=3) as xpool,
            tc.tile_pool(name="zst", bufs=3) as zpool,
            tc.tile_pool(name="bsp", bufs=3) as bspool,
            tc.tile_pool(name="prods", bufs=6) as ppool,
            tc.tile_pool(name="outs", bufs=3) as opool,
            tc.tile_pool(name="pa", bufs=2, space="PSUM") as pa_pool,
            tc.tile_pool(name="pb", bufs=3, space="PSUM") as pb_pool,
            tc.tile_pool(name="po", bufs=1, space="PSUM") as po_pool,
        ):
            cws = cpool.tile([D, CONST_W], BF16)
            nc.sync.dma_start(cws[:], cw[:])
            p2s = cws[:, 0:KR]
            q2s = cws[:, KR : 2 * KR]
            wts = cws[:, 2 * KR : 2 * KR + K]
            sels = cws[:, 2 * KR + K : 2 * KR + K + SELW]

            # single shared PSUM bank for outT; blocks alternate halves
            po_all = po_pool.tile([128, BLK], F32, name="po_all", tag="outp")

            # pending sel work: (j, c, prod_view) emitted SEL_SKEW pairs late
            pending = []
            # blocks whose last sels are emitted; osb pending
            osb_pending = []

            def outp_of(j):
                return po_all[64 * (j % 2) : 64 * (j % 2) + 64, :]

            def emit_sel(j, c, prod_view):
                # 32-row strip: consecutive chunks use different col-groups.
                # NOTE: skip_group_check -- CoreSim's zero-region tracker
                # false-positives on strip accumulation; HW per-element
                # has_written semantics are exact (lin start=True clears the
                # written partitions' bank rows, strips then accumulate).
                base = 64 * (j % 2) + 32 * (c % 2)
                strip = po_all[base : base + 32, :]
                nc.tensor.matmul(
                    strip,
                    sels[:, c * 32 : (c + 1) * 32],
                    prod_view,
                    start=False,
                    stop=(c >= NCHUNK - 2),
                    skip_group_check=True,
                    tile_position=(0, base),
                )

            def flush_pending(upto):
                # emit queued sel MMs while more than `upto` remain
                while len(pending) > upto:
                    j, c, pv = pending.pop(0)
                    was_last = c >= NCHUNK - 2
                    emit_sel(j, c, pv)
                    if was_last and c == NCHUNK - 1:
                        osb_pending.append(j)
                        flush_osb()

            def flush_osb():
                while osb_pending:
                    j = osb_pending.pop(0)
                    osb = opool.tile([K, BLK], F32, name=f"osb{j}", tag="osb")
                    nc.vector.tensor_copy(osb[:], outp_of(j))
                    nc.sync.dma_start(out[:, ts(j, BLK)], osb[:])

            def do_superblock(js, zts, sts):
                """js: list of block indices sharing each stationary load.
                Each P/Q chunk is LDW'd once and streams all blocks' tokens."""
                for p in range(NPAIR):
                    c0, c1 = 2 * p, 2 * p + 1
                    # interleave B and A sub-rounds (stationary still shared
                    # across blocks) so the ACT evac chain is fed evenly
                    # instead of in one burst
                    bss, pas, prods = [], [], []
                    for bi, j in enumerate(js):
                        b0 = pb_pool.tile([128, BLK], F32, name=f"b{j}_{c0}", tag="B")
                        nc.tensor.matmul(
                            b0[:], q2s[:, ts(c0, 128)], sts[bi],
                            start=True, stop=True,
                        )
                        bs = bspool.tile(
                            [128, 2, BLK], BF16, name=f"bs{j}_{p}", tag="bs"
                        )
                        nc.scalar.copy(bs[:, 0, :], b0[:])
                        bss.append(bs)
                    for bi, j in enumerate(js):
                        pa = pa_pool.tile(
                            [128, 2, BLK], F32, name=f"a{j}_{p}", tag="A"
                        )
                        nc.tensor.matmul(
                            pa[:, 0, :], p2s[:, ts(c0, 128)], zts[bi],
                            start=True, stop=True,
                        )
                        pas.append(pa)
                    for bi, j in enumerate(js):
                        b1 = pb_pool.tile([128, BLK], F32, name=f"b{j}_{c1}", tag="B")
                        nc.tensor.matmul(
                            b1[:], q2s[:, ts(c1, 128)], sts[bi],
                            start=True, stop=True,
                        )
                        nc.scalar.copy(bss[bi][:, 1, :], b1[:])
                    for bi, j in enumerate(js):
                        nc.tensor.matmul(
                            pas[bi][:, 1, :], p2s[:, ts(c1, 128)], zts[bi],
                            start=True, stop=True,
                        )
                        prod = ppool.tile(
                            [128, 2, BLK], BF16, name=f"prod{j}_{p}", tag="prod"
                        )
                        nc.vector.tensor_mul(prod[:], pas[bi][:], bss[bi][:])
                        prods.append(prod)
                    if p == 1:
                        # lins BEFORE round 0's sels are flushed (below), and
                        # after round 0's flush drained the previous
                        # superblock's trailing sels + osb on these po halves
                        for bi, j in enumerate(js):
                            nc.tensor.matmul(
                                outp_of(j), wts, zts[bi], start=True, stop=False,
                                skip_group_check=True,
                                tile_position=(0, 64 * (j % 2)),
                            )
                    # uniform 1-round skew: drain the previous round's sels
                    # (each round leaves exactly one round's worth pending)
                    flush_pending(0)
                    for bi, j in enumerate(js):
                        pending.append((j, c0, prods[bi][:, 0, :]))
                        pending.append((j, c1, prods[bi][:, 1, :]))

            for jj in range(n_tokens // xblk):
                x1c = xpool.tile([D, xblk], BF16, name=f"x1c{jj}", tag="x1c")
                nc.sync.dma_start(x1c[:], x1t[:, ts(jj, xblk)])
                x2c = xpool.tile([D, xblk], BF16, name=f"x2c{jj}", tag="x2c")
                nc.sync.dma_start(x2c[:], x2t[:, ts(jj, xblk)])

                zs = zpool.tile([D, 2, xblk], BF16, name=f"zs{jj}", tag="zs")
                # first superblock on DVE (fast, shortens the startup serial
                # chain); steady state on the otherwise-idle GPSIMD
                zeng = nc.vector if jj == 0 else nc.gpsimd
                zeng.tensor_sub(zs[:, 0, :], x1c[:], x2c[:])
                zeng.tensor_add(zs[:, 1, :], x1c[:], x2c[:])

                nb = xblk // BLK
                js = [jj * nb + h for h in range(nb)]
                do_superblock(
                    js,
                    [zs[:, 0, ts(h, BLK)] for h in range(nb)],
                    [zs[:, 1, ts(h, BLK)] for h in range(nb)],
                )

            flush_pending(0)
            flush_osb()

    nc.finalize()
    return nc


def _perm():
    # out-row for k = 8c+t is  newk = 32*(c%2) + 8*(c//2) + t
    perm = np.zeros(K, dtype=np.int64)
    for c in range(NCHUNK):
        for t in range(8):
            perm[8 * c + t] = 32 * (c % 2) + 8 * (c // 2) + t
    return perm


def _make_sel():
    # sel_c maps kr-partition p to strip-local row 8*(c//2) + p//16
    sel = np.zeros((NCHUNK, 128, 32), dtype=np.float32)
    for c in range(NCHUNK):
        for p in range(128):
            sel[c, p, 8 * (c // 2) + p // 16] = 1.0
    return sel.transpose(1, 0, 2).reshape(128, NCHUNK * 32)


def _shard_and_pack(x1, x2, W_lin, P, Q):
    p2 = P.transpose(1, 0, 2).reshape(D, KR)
    q2 = Q.transpose(1, 0, 2).reshape(D, KR)
    wt = np.ascontiguousarray(W_lin.T)[:, np.argsort(_perm())]
    cwv = np.concatenate([p2, q2, wt, _make_sel()], axis=1).astype(
        ml_dtypes.bfloat16
    )
    assert cwv.shape == (D, CONST_W)

    in_maps = []
    for b in range(N_CORES):
        in_maps.append(
            {
                "x1t": np.ascontiguousarray(x1[b].T).astype(ml_dtypes.bfloat16),
                "x2t": np.ascontiguousarray(x2[b].T).astype(ml_dtypes.bfloat16),
                "cw": cwv,
            }
        )
    return in_maps


def postprocess(out_raw):
    """Per-core raw DRAM output [K, n] (permuted rows) -> [n, K] natural."""
    return np.ascontiguousarray(out_raw[_perm(), :].T)


def kernel(x1, x2, W_lin, P, Q):
    assert x1.shape == (N_CORES, 16384, D) and x2.shape == x1.shape
    nc = build_bass(16384)
    in_maps = _shard_and_pack(x1, x2, W_lin, P, Q)
    res = run_bass_kernel_spmd(nc, in_maps, core_ids=list(range(N_CORES)))
    out = np.stack(
        [postprocess(res.results[b]["out"]) for b in range(N_CORES)], axis=0
    )
    return out.astype(np.float32)
